# revision 36
# baseline (speedup 1.0000x reference)
# ContextQueryAttention (BiDAF-style) Trainium2 Bass/Tile kernel.
#
# Full-input contract: kernel(**inputs) takes the full arrays
#   context [32, 2048, 128] f32, query [32, 128, 128] f32,
#   w [384] f32, query_mask [32, 128] i32
# and returns out [32, 2048, 512] f32.
#
# Sharding: batch B=32 split 4-per-core across 8 NeuronCores (pure data
# parallel, no collectives).
#
# Math (per batch, C=2048, Q=128, D=128):
#   S[c,q] = ctx[c]@w1 + query[q]@w2 + (ctx[c]*w3)@query[q]
#          = alpha[c] + beta[q] + G[c,q]
#   a = softmax_q(S + maskadd);  c2q = a @ query
#   m[c] = max_q(S + maskadd);   b = softmax_c(m); q2c = b @ ctx
#   out = [ctx | c2q | ctx*c2q | ctx*q2c]
#
# Design notes (cost-model driven):
#  * alpha[c] cancels in softmax_q -> row softmax runs on T = G + beta'
#    (beta' = beta + mask_add) fused into the ACT exp bias in [q, c] layout.
#  * |S| = O(5), so exp() without max-subtraction is exact to fp32 roundoff.
#  * E^T = exp(T^T) is stored in *bf16*: the c2q matmuls and the E
#    transposes then run at 1 cyc/row on the PE (vs 4 for fp32).  rel-err
#    budget is 2e-2; bf16 E costs ~3e-4.
#  * max_q E per c-tile via PE-transpose of E^T; 4 tiles transposed into
#    one PSUM bank and reduced with a single 3D reduce_max.
#  * u = sum_c e_m[c]*ctx[c] computed transposed: stationary ctx tile,
#    moving e_m column -> N=1 matmuls (~free on PE).
#  * One SBUF assembly tile per batch [128, 16*512]; ctx is DMA'd straight
#    into its first column block; 2 stores/batch (cols 0:384 and 384:512).
#    All loads are issued before any store on the SP queue so a waiting
#    store never blocks a later batch's load.
#  * Elementwise work is spread: exp/scales on ACT, muls/reduces on DVE,
#    ctxT copies + half the out4 muls on gpsimd (Pool).
#
# PSUM (8 banks): big 2 (ctx transposes / G / small head+tail) + etr 2
# (E-transpose groups) + cq 3 (c2q results, 2 tiles packed per bank) +
# u 1 (accumulation chain owns its bank).

import numpy as np

C = 2048
Q = 128
D = 128
B_TOTAL = 32
N_CORES = 8
B_LOCAL = B_TOTAL // N_CORES  # 4
N_CT = C // 128  # 16 c-tiles per batch
N_G = 4  # groups of 4 c-tiles

_compiled = None


def _build():
    import concourse.bacc as bacc
    import concourse.tile as tile
    import concourse.mybir as mybir
    from concourse import masks

    f32 = mybir.dt.float32
    i32 = mybir.dt.int32

    nc = bacc.Bacc(
        "TRN2",
        target_bir_lowering=False,
        debug=False,
        num_devices=N_CORES,
    )

    ctx_d = nc.dram_tensor("context", [B_LOCAL, C, D], f32, kind="ExternalInput").ap()
    qry_d = nc.dram_tensor("query", [B_LOCAL, Q, D], f32, kind="ExternalInput").ap()
    w_d = nc.dram_tensor("w", [3 * D], f32, kind="ExternalInput").ap()
    msk_d = nc.dram_tensor("query_mask", [B_LOCAL, Q], i32, kind="ExternalInput").ap()
    out_d = nc.dram_tensor("out", [B_LOCAL, C, 4 * D], f32, kind="ExternalOutput").ap()

    with tile.TileContext(nc) as tc:
        _kernel_body(tc, out_d, ctx_d, qry_d, w_d, msk_d, mybir, masks)

    nc.compile()
    return nc


def _kernel_body(tc, out_d, ctx_d, qry_d, w_d, msk_d, mybir, masks):
    from contextlib import ExitStack

    nc = tc.nc
    f32 = mybir.dt.float32
    bf16 = mybir.dt.bfloat16
    f32r = mybir.dt.float32r
    i32 = mybir.dt.int32
    AFT = mybir.ActivationFunctionType
    Alu = mybir.AluOpType
    AX = mybir.AxisListType.X

    es = ExitStack()
    with es:
        # ---- pools ----
        consts = es.enter_context(tc.tile_pool(name="consts", bufs=1))
        outp = es.enter_context(tc.tile_pool(name="outp", bufs=4))
        bigs = es.enter_context(tc.tile_pool(name="bigs", bufs=2))
        meds = es.enter_context(tc.tile_pool(name="meds", bufs=2))
        cols = es.enter_context(tc.tile_pool(name="cols", bufs=8))
        ps_big = es.enter_context(tc.tile_pool(name="ps_big", bufs=2, space="PSUM"))
        ps_etr = es.enter_context(tc.tile_pool(name="ps_etr", bufs=2, space="PSUM"))
        ps_cq = es.enter_context(tc.tile_pool(name="ps_cq", bufs=3, space="PSUM"))
        ps_tail = es.enter_context(tc.tile_pool(name="ps_tail", bufs=1, space="PSUM"))

        # ---- constants ----
        # One SWDGE DMA for all of w (no HWDGE contention with SP loads,
        # single ~1us generation) -- issued before the identity builders so
        # w3 is ready for qw3T as early as possible.
        wcols = consts.tile([128, 3], f32)
        nc.gpsimd.dma_start(out=wcols[:], in_=w_d.rearrange("(k d) -> d k", k=3))
        ident = consts.tile([128, 128], f32)
        masks.make_identity(nc, ident[:])
        ident_bf = consts.tile([128, 128], bf16)
        masks.make_identity(nc, ident_bf[:])
        w1_col = consts.tile([128, 1], bf16)
        nc.vector.tensor_copy(w1_col[:], wcols[:, 0:1])  # f32 -> bf16
        w2_col = wcols[:, 1:2]
        w3_col = wcols[:, 2:3]
        ones_col = consts.tile([128, 1], f32)
        nc.vector.memset(ones_col[:], 1.0)
        ones_row_bf = consts.tile([1, 128], bf16)
        nc.vector.memset(ones_row_bf[:], 1.0)

        msk3 = msk_d.rearrange("b q -> b q ()")
        # c-tile i holds context rows c = i*128 + p
        ctx_v = ctx_d.rearrange("b (i p) d -> b p i d", p=128)
        out_v = out_d.rearrange("b (i p) f -> b p i f", p=128)

        # ---------- loads (SP queue) ----------
        # Small loads + the first two batches' ctx go up front (no waits);
        # ctx for batches 2/3 is deferred into the store stream so an
        # early-batch store never queues behind a load it doesn't need.
        gts = []
        gvs = []
        qfs = []
        mks = []
        for b in range(B_LOCAL):
            qf = meds.tile([128, 128], f32, tag="qf", bufs=4)
            mk = cols.tile([128, 1], i32, tag="mask", bufs=4)
            gt = outp.tile([128, N_CT * 512], f32, tag="out")
            gv = gt.rearrange("p (i f) -> p i f", i=N_CT)
            gts.append(gt)
            gvs.append(gv)
            qfs.append(qf)
            mks.append(mk)

        def load_qm(b):
            nc.sync.dma_start(out=qfs[b][:], in_=qry_d[b])
            nc.sync.dma_start(out=mks[b][:], in_=msk3[b])

        def load_ctx(b, quarters=True):
            # quartered: group g of 4 c-tiles arrives separately so batch-0
            # transposes can start after the first 256 KiB.
            if quarters:
                for g in range(N_G):
                    nc.sync.dma_start(
                        out=gvs[b][:, 4 * g : 4 * g + 4, 0:128],
                        in_=ctx_v[b][:, 4 * g : 4 * g + 4, :],
                    )
            else:
                nc.sync.dma_start(out=gvs[b][:, :, 0:128], in_=ctx_v[b])

        nc.sync.dma_start(
            out=gvs[0][:, 0:4, 0:128], in_=ctx_v[0][:, 0:4, :]
        )
        nc.sync.dma_start(
            out=gvs[0][:, 4:N_CT, 0:128], in_=ctx_v[0][:, 4:N_CT, :]
        )
        load_qm(0)
        load_ctx(1, quarters=False)
        load_qm(1)
        load_qm(2)
        load_qm(3)
        load_ctx(2, quarters=False)
        # out[:, :, 0:128] == ctx: ship it DRAM->DRAM on the SWDGE queue.
        # No compute dependency, so these fill every DMA idle window.
        for b in range(B_LOCAL):
            nc.gpsimd.dma_start(out=out_v[b][:, :, 0:128], in_=ctx_v[b])

        for b in range(B_LOCAL):
            gv = gvs[b]

            def ctx_blk(i):
                return gv[:, i, 0:128]

            def o_blk(i, k):
                return gv[:, i, k * 128 : (k + 1) * 128]

            # ---------- query prep ----------
            rhs_aug = meds.tile([128, 129], bf16, tag="rhs")
            nc.vector.tensor_copy(rhs_aug[:, 0:128], qfs[b][:])  # f32 -> bf16
            nc.vector.memset(rhs_aug[:, 128:129], 1.0)
            madd_col = cols.tile([128, 1], f32, tag="madd")
            nc.vector.tensor_copy(madd_col[:], mks[b][:])  # int -> float cast
            nc.vector.tensor_scalar(
                madd_col[:], madd_col[:], 1.0, 1.0e9, op0=Alu.subtract, op1=Alu.mult
            )

            # headA: qT (cols 0:128) + beta col (col 128)
            headA = ps_big.tile([128, 512], f32, tag="big")
            nc.tensor.transpose(headA[:, 0:128], qfs[b][:], ident[:])
            qT = meds.tile([128, 128], f32, tag="qT")
            nc.vector.tensor_copy(qT[:], headA[:, 0:128])
            qw3T = meds.tile([128, 128], bf16, tag="qw3T")
            nc.vector.tensor_scalar_mul(qw3T[:], qT[:], w3_col[:])
            nc.tensor.matmul(
                headA[:, 128:129], qT[:], w2_col[:], start=True, stop=True
            )
            beta_col = cols.tile([128, 1], f32, tag="beta")
            nc.vector.tensor_add(beta_col[:], madd_col[:], headA[:, 128:129])

            # per-batch tiles
            ctxT = bigs.tile([128, C], bf16, tag="ctxT")
            e_t = bigs.tile([128, C], bf16, tag="et")
            e_alpha = meds.tile([128, N_CT], f32, tag="ealpha")
            e_m = meds.tile([128, N_CT], f32, tag="em")
            # tail bank: u chain col 0, zb col 1, q2c row 2:130, bc 130:258
            tail_ps = ps_tail.tile([128, 512], f32, tag="tail")

            def stage_transposes(g):
                tr_ps = ps_big.tile([128, 512], f32, tag="big")
                for j in range(4):
                    nc.tensor.transpose(
                        tr_ps[:, j * 128 : (j + 1) * 128], ctx_blk(g * 4 + j), ident[:]
                    )
                if g % 2 == 0:
                    nc.scalar.copy(ctxT[:, g * 512 : (g + 1) * 512], tr_ps[:])
                else:
                    nc.vector.tensor_copy(ctxT[:, g * 512 : (g + 1) * 512], tr_ps[:])

            # ---------- group-pipelined main loop ----------
            # Per group: G -> exp -> cq/etr -> scales/out3 -> store, with the
            # next group's ctx transposes staged one iteration ahead so the
            # PSUM->SBUF copy and exp latency never stall the PE.
            stage_transposes(0)
            for g in range(N_G):
                # G^T for this group (ctxT[g] copied last iteration)
                st_ps = ps_big.tile([128, 512], f32, tag="big")
                nc.tensor.matmul(
                    st_ps[:],
                    qw3T[:],
                    ctxT[:, g * 512 : (g + 1) * 512],
                    start=True,
                    stop=True,
                )
                if g + 1 < N_G:
                    stage_transposes(g + 1)
                nc.scalar.activation(
                    out=e_t[:, g * 512 : (g + 1) * 512],
                    in_=st_ps[:],
                    func=AFT.Exp,
                    bias=beta_col[:],
                    scale=1.0,
                )
                # cq PSUM allocs; alpha columns ride in cq2a cols 258:262
                cq2a = ps_cq.tile([128, 262], f32, tag="cq")
                for jj in range(4):
                    i = 4 * g + jj
                    nc.tensor.matmul(
                        cq2a[:, 258 + jj : 259 + jj],
                        ctxT[:, i * 128 : (i + 1) * 128],
                        w1_col[:],
                        start=True,
                        stop=True,
                    )
                # u chain for the previous group (e_m ready by then)
                if g > 0:
                    for i in range(4 * (g - 1), 4 * g):
                        nc.tensor.matmul(
                            tail_ps[:, 0:1],
                            ctx_blk(i),
                            e_m[:, i : i + 1],
                            start=(i == 0),
                            stop=False,
                        )
                etr = ps_etr.tile([128, 512], bf16, tag="etr")
                rzs = {}
                cq_slots = {}
                cq2 = cq2a
                for j in range(4):
                    i = 4 * g + j
                    if j == 2:
                        cq2 = ps_cq.tile([128, 262], f32, tag="cq")
                    cqs = cq2[:, 129 * (j % 2) : 129 * (j % 2) + 129]
                    cq_slots[j] = cqs
                    et_sl = e_t[:, i * 128 : (i + 1) * 128]
                    nc.tensor.matmul(cqs, et_sl, rhs_aug[:], start=True, stop=True)
                    nc.tensor.transpose(
                        etr[:, j * 128 : (j + 1) * 128], et_sl, ident_bf[:]
                    )
                    if j % 2 == 1:
                        # one reciprocal per pair: Z cols sit at 128 and 257
                        rz2 = cols.tile([128, 2], f32, tag="rz")
                        zv = cq2[:, 0:258].rearrange("p (k n) -> p k n", k=2)[
                            :, :, 128
                        ]
                        nc.vector.reciprocal(rz2[:], zv)
                        rzs[j - 1] = rz2[:, 0:1]
                        rzs[j] = rz2[:, 1:2]
                for j in range(4):
                    i = 4 * g + j
                    # c2q = (E @ [qry|1]) / Z  (ACT copy with per-partition scale)
                    nc.scalar.activation(
                        out=o_blk(i, 1),
                        in_=cq_slots[j][:, 0:128],
                        func=AFT.Copy,
                        scale=rzs[j],
                    )
                # e^alpha for this group's 4 tiles (not on the store path)
                nc.scalar.activation(
                    out=e_alpha[:, 4 * g : 4 * g + 4],
                    in_=cq2a[:, 258:262],
                    func=AFT.Exp,
                )
                # row max over q of the 4 transposed tiles, then e_m
                maxE = cols.tile([128, 4], f32, tag="maxE")
                nc.vector.reduce_max(
                    out=maxE[:], in_=etr.rearrange("p (j q) -> p j q", j=4), axis=AX
                )
                nc.vector.tensor_mul(
                    e_m[:, 4 * g : 4 * g + 4], e_alpha[:, 4 * g : 4 * g + 4], maxE[:]
                )
                # out3 = ctx * c2q (SBUF-only: gpsimd can help)
                for j in range(4):
                    i = 4 * g + j
                    eng = nc.gpsimd if j >= 2 else nc.vector
                    eng.tensor_mul(o_blk(i, 2), ctx_blk(i), o_blk(i, 1))
                # this group's [c2q | ctx*c2q] is final: ship it
                # (batch 0 group 0 in two halves so DMA starts sooner)
                if b == 0 and g == 0:
                    nc.sync.dma_start(
                        out=out_v[b][:, 0:2, 128:384], in_=gv[:, 0:2, 128:384]
                    )
                    nc.sync.dma_start(
                        out=out_v[b][:, 2:4, 128:384], in_=gv[:, 2:4, 128:384]
                    )
                else:
                    nc.sync.dma_start(
                        out=out_v[b][:, 4 * g : 4 * g + 4, 128:384],
                        in_=gv[:, 4 * g : 4 * g + 4, 128:384],
                    )
            for i in range(4 * (N_G - 1), N_CT):
                nc.tensor.matmul(
                    tail_ps[:, 0:1],
                    ctx_blk(i),
                    e_m[:, i : i + 1],
                    start=False,
                    stop=(i == N_CT - 1),
                )

            # ---------- q2c epilogue ----------
            zsum = cols.tile([128, 1], f32, tag="zsum")
            nc.vector.reduce_sum(out=zsum[:], in_=e_m[:], axis=AX)
            nc.tensor.matmul(
                tail_ps[0:1, 1:2], zsum[:], ones_col[:], start=True, stop=True
            )
            u_sb = cols.tile([128, 1], f32, tag="usb")
            nc.vector.tensor_copy(u_sb[:], tail_ps[:, 0:1])
            nc.tensor.transpose(tail_ps[0:1, 2:130], u_sb[:], ident[:])
            rzb = cols.tile([1, 1], f32, tag="rzb")
            nc.vector.reciprocal(rzb[:], tail_ps[0:1, 1:2])
            q2c_row = cols.tile([1, 128], bf16, tag="q2crow")
            nc.scalar.activation(
                out=q2c_row[:], in_=tail_ps[0:1, 2:130], func=AFT.Copy, scale=rzb[:]
            )
            nc.tensor.matmul(
                tail_ps[:, 130:258], ones_row_bf[:], q2c_row[:], start=True, stop=True
            )
            q2c_sb = meds.tile([128, 128], f32, tag="q2csb")
            nc.vector.tensor_copy(q2c_sb[:], tail_ps[:, 130:258])

            # ---------- out4 + remaining stores (4 pieces) ----------
            # 3 DVE + 1 gpsimd mul per piece; each piece ships on completion
            for piece in range(4):
                for j in range(4):
                    i = 4 * piece + j
                    eng = nc.gpsimd if j == 1 else nc.vector
                    eng.tensor_mul(o_blk(i, 3), ctx_blk(i), q2c_sb[:])
                nc.sync.dma_start(
                    out=out_v[b][:, 4 * piece : 4 * piece + 4, 384:512],
                    in_=gv[:, 4 * piece : 4 * piece + 4, 384:512],
                )
            # ctx3 rides behind batch 1's stores (not needed until then)
            if b == 1:
                load_ctx(3, quarters=False)


def kernel(**inputs):
    global _compiled
    from concourse.bass_utils import run_bass_kernel_spmd

    context = np.ascontiguousarray(inputs["context"], dtype=np.float32)
    query = np.ascontiguousarray(inputs["query"], dtype=np.float32)
    w = np.ascontiguousarray(inputs["w"], dtype=np.float32)
    qmask = np.ascontiguousarray(inputs["query_mask"], dtype=np.int32)

    if _compiled is None:
        _compiled = _build()
    nc = _compiled

    core_ids = list(range(N_CORES))
    in_maps = []
    for k in core_ids:
        sl = slice(k * B_LOCAL, (k + 1) * B_LOCAL)
        in_maps.append(
            {
                "context": context[sl],
                "query": query[sl],
                "w": w,
                "query_mask": qmask[sl],
            }
        )

    res = run_bass_kernel_spmd(nc, in_maps, core_ids)
    outs = [res.results[k]["out"] for k in range(N_CORES)]
    return np.concatenate(outs, axis=0)


# revision 37
# speedup vs baseline: 1.0478x; 1.0478x over previous
# ContextQueryAttention (BiDAF-style) Trainium2 Bass/Tile kernel.
#
# Full-input contract: kernel(**inputs) takes the full arrays
#   context [32, 2048, 128] f32, query [32, 128, 128] f32,
#   w [384] f32, query_mask [32, 128] i32
# and returns out [32, 2048, 512] f32.
#
# Sharding: batch B=32 split 4-per-core across 8 NeuronCores (pure data
# parallel, no collectives).
#
# Math (per batch, C=2048, Q=128, D=128):
#   S[c,q] = ctx[c]@w1 + query[q]@w2 + (ctx[c]*w3)@query[q]
#          = alpha[c] + beta[q] + G[c,q]
#   a = softmax_q(S + maskadd);  c2q = a @ query
#   m[c] = max_q(S + maskadd);   b = softmax_c(m); q2c = b @ ctx
#   out = [ctx | c2q | ctx*c2q | ctx*q2c]
#
# Design notes (cost-model driven):
#  * alpha[c] cancels in softmax_q -> row softmax runs on T = G + beta'
#    (beta' = beta + mask_add) fused into the ACT exp bias in [q, c] layout.
#  * |S| = O(5), so exp() without max-subtraction is exact to fp32 roundoff.
#  * E^T = exp(T^T) is stored in *bf16*: the c2q matmuls and the E
#    transposes then run at 1 cyc/row on the PE (vs 4 for fp32).  rel-err
#    budget is 2e-2; bf16 E costs ~3e-4.
#  * max_q E per c-tile via PE-transpose of E^T; 4 tiles transposed into
#    one PSUM bank and reduced with a single 3D reduce_max.
#  * u = sum_c e_m[c]*ctx[c] computed transposed: stationary ctx tile,
#    moving e_m column -> N=1 matmuls (~free on PE).
#  * One SBUF assembly tile per batch [128, 16*512]; ctx is DMA'd straight
#    into its first column block; 2 stores/batch (cols 0:384 and 384:512).
#    All loads are issued before any store on the SP queue so a waiting
#    store never blocks a later batch's load.
#  * Elementwise work is spread: exp/scales on ACT, muls/reduces on DVE,
#    ctxT copies + half the out4 muls on gpsimd (Pool).
#
# PSUM (8 banks): big 2 (ctx transposes / G / small head+tail) + etr 2
# (E-transpose groups) + cq 3 (c2q results, 2 tiles packed per bank) +
# u 1 (accumulation chain owns its bank).

import numpy as np

C = 2048
Q = 128
D = 128
B_TOTAL = 32
N_CORES = 8
B_LOCAL = B_TOTAL // N_CORES  # 4
N_CT = C // 128  # 16 c-tiles per batch
N_G = 4  # groups of 4 c-tiles

_compiled = None


def _build():
    import concourse.bacc as bacc
    import concourse.tile as tile
    import concourse.mybir as mybir
    from concourse import masks

    f32 = mybir.dt.float32
    i32 = mybir.dt.int32

    nc = bacc.Bacc(
        "TRN2",
        target_bir_lowering=False,
        debug=False,
        num_devices=N_CORES,
    )

    ctx_d = nc.dram_tensor("context", [B_LOCAL, C, D], f32, kind="ExternalInput").ap()
    qry_d = nc.dram_tensor("query", [B_LOCAL, Q, D], f32, kind="ExternalInput").ap()
    w_d = nc.dram_tensor("w", [3 * D], f32, kind="ExternalInput").ap()
    msk_d = nc.dram_tensor("query_mask", [B_LOCAL, Q], i32, kind="ExternalInput").ap()
    out_d = nc.dram_tensor("out", [B_LOCAL, C, 4 * D], f32, kind="ExternalOutput").ap()

    with tile.TileContext(nc) as tc:
        _kernel_body(tc, out_d, ctx_d, qry_d, w_d, msk_d, mybir, masks)

    nc.compile()
    return nc


def _kernel_body(tc, out_d, ctx_d, qry_d, w_d, msk_d, mybir, masks):
    from contextlib import ExitStack

    nc = tc.nc
    f32 = mybir.dt.float32
    bf16 = mybir.dt.bfloat16
    f32r = mybir.dt.float32r
    i32 = mybir.dt.int32
    AFT = mybir.ActivationFunctionType
    Alu = mybir.AluOpType
    AX = mybir.AxisListType.X

    es = ExitStack()
    with es:
        # ---- pools ----
        consts = es.enter_context(tc.tile_pool(name="consts", bufs=1))
        outp = es.enter_context(tc.tile_pool(name="outp", bufs=4))
        bigs = es.enter_context(tc.tile_pool(name="bigs", bufs=2))
        meds = es.enter_context(tc.tile_pool(name="meds", bufs=2))
        cols = es.enter_context(tc.tile_pool(name="cols", bufs=8))
        ps_big = es.enter_context(tc.tile_pool(name="ps_big", bufs=2, space="PSUM"))
        ps_etr = es.enter_context(tc.tile_pool(name="ps_etr", bufs=2, space="PSUM"))
        ps_cq = es.enter_context(tc.tile_pool(name="ps_cq", bufs=3, space="PSUM"))
        ps_tail = es.enter_context(tc.tile_pool(name="ps_tail", bufs=1, space="PSUM"))

        # ---- constants ----
        # One SWDGE DMA for all of w (no HWDGE contention with SP loads,
        # single ~1us generation) -- issued before the identity builders so
        # w3 is ready for qw3T as early as possible.
        wcols = consts.tile([128, 3], f32)
        nc.gpsimd.dma_start(out=wcols[:], in_=w_d.rearrange("(k d) -> d k", k=3))
        ident = consts.tile([128, 128], f32)
        masks.make_identity(nc, ident[:])
        ident_bf = consts.tile([128, 128], bf16)
        masks.make_identity(nc, ident_bf[:])
        w1_col = consts.tile([128, 1], bf16)
        nc.vector.tensor_copy(w1_col[:], wcols[:, 0:1])  # f32 -> bf16
        w2_col = wcols[:, 1:2]
        w3_col = wcols[:, 2:3]
        ones_col = consts.tile([128, 1], f32)
        nc.vector.memset(ones_col[:], 1.0)
        ones_row_bf = consts.tile([1, 128], bf16)
        nc.vector.memset(ones_row_bf[:], 1.0)

        msk3 = msk_d.rearrange("b q -> b q ()")
        # c-tile i holds context rows c = i*128 + p
        ctx_v = ctx_d.rearrange("b (i p) d -> b p i d", p=128)
        out_v = out_d.rearrange("b (i p) f -> b p i f", p=128)

        # ---------- loads (SP queue) ----------
        # Small loads + the first two batches' ctx go up front (no waits);
        # ctx for batches 2/3 is deferred into the store stream so an
        # early-batch store never queues behind a load it doesn't need.
        gts = []
        gvs = []
        qfs = []
        mks = []
        for b in range(B_LOCAL):
            qf = meds.tile([128, 128], f32, tag="qf", bufs=4)
            mk = cols.tile([128, 1], i32, tag="mask", bufs=4)
            gt = outp.tile([128, N_CT * 512], f32, tag="out")
            gv = gt.rearrange("p (i f) -> p i f", i=N_CT)
            gts.append(gt)
            gvs.append(gv)
            qfs.append(qf)
            mks.append(mk)

        def load_qm(b):
            nc.sync.dma_start(out=qfs[b][:], in_=qry_d[b])
            nc.sync.dma_start(out=mks[b][:], in_=msk3[b])

        def load_ctx(b, quarters=True):
            # quartered: group g of 4 c-tiles arrives separately so batch-0
            # transposes can start after the first 256 KiB.
            if quarters:
                for g in range(N_G):
                    nc.sync.dma_start(
                        out=gvs[b][:, 4 * g : 4 * g + 4, 0:128],
                        in_=ctx_v[b][:, 4 * g : 4 * g + 4, :],
                    )
            else:
                nc.sync.dma_start(out=gvs[b][:, :, 0:128], in_=ctx_v[b])

        nc.sync.dma_start(
            out=gvs[0][:, 0:4, 0:128], in_=ctx_v[0][:, 0:4, :]
        )
        nc.sync.dma_start(out=qfs[0][:], in_=qry_d[0])
        nc.sync.dma_start(
            out=gvs[0][:, 4:N_CT, 0:128], in_=ctx_v[0][:, 4:N_CT, :]
        )
        nc.sync.dma_start(out=mks[0][:], in_=msk3[0])
        load_ctx(1, quarters=False)
        load_qm(1)
        load_qm(2)
        load_qm(3)
        load_ctx(2, quarters=False)
        # out[:, :, 0:128] == ctx: ship it DRAM->DRAM on the SWDGE queue.
        # No compute dependency, so these fill every DMA idle window.
        for b in range(B_LOCAL):
            nc.gpsimd.dma_start(out=out_v[b][:, :, 0:128], in_=ctx_v[b])

        for b in range(B_LOCAL):
            gv = gvs[b]

            def ctx_blk(i):
                return gv[:, i, 0:128]

            def o_blk(i, k):
                return gv[:, i, k * 128 : (k + 1) * 128]

            # ---------- query prep ----------
            rhs_aug = meds.tile([128, 129], bf16, tag="rhs")
            nc.vector.tensor_copy(rhs_aug[:, 0:128], qfs[b][:])  # f32 -> bf16
            nc.vector.memset(rhs_aug[:, 128:129], 1.0)
            madd_col = cols.tile([128, 1], f32, tag="madd")
            nc.vector.tensor_copy(madd_col[:], mks[b][:])  # int -> float cast
            nc.vector.tensor_scalar(
                madd_col[:], madd_col[:], 1.0, 1.0e9, op0=Alu.subtract, op1=Alu.mult
            )

            # headA: qT (cols 0:128) + beta col (col 128)
            headA = ps_big.tile([128, 512], f32, tag="big")
            nc.tensor.transpose(headA[:, 0:128], qfs[b][:], ident[:])
            qT = meds.tile([128, 128], f32, tag="qT")
            nc.vector.tensor_copy(qT[:], headA[:, 0:128])
            qw3T = meds.tile([128, 128], bf16, tag="qw3T")
            nc.vector.tensor_scalar_mul(qw3T[:], qT[:], w3_col[:])
            nc.tensor.matmul(
                headA[:, 128:129], qT[:], w2_col[:], start=True, stop=True
            )
            beta_col = cols.tile([128, 1], f32, tag="beta")
            nc.vector.tensor_add(beta_col[:], madd_col[:], headA[:, 128:129])

            # per-batch tiles
            ctxT = bigs.tile([128, C], bf16, tag="ctxT")
            e_t = bigs.tile([128, C], bf16, tag="et")
            e_alpha = meds.tile([128, N_CT], f32, tag="ealpha")
            e_m = meds.tile([128, N_CT], f32, tag="em")
            # tail bank: u chain col 0, zb col 1, q2c row 2:130, bc 130:258
            tail_ps = ps_tail.tile([128, 512], f32, tag="tail")

            def stage_transposes(g):
                tr_ps = ps_big.tile([128, 512], f32, tag="big")
                for j in range(4):
                    nc.tensor.transpose(
                        tr_ps[:, j * 128 : (j + 1) * 128], ctx_blk(g * 4 + j), ident[:]
                    )
                if g % 2 == 0:
                    nc.scalar.copy(ctxT[:, g * 512 : (g + 1) * 512], tr_ps[:])
                else:
                    nc.vector.tensor_copy(ctxT[:, g * 512 : (g + 1) * 512], tr_ps[:])

            # ---------- group-pipelined main loop ----------
            # Per group: G -> exp -> cq/etr -> scales/out3 -> store, with the
            # next group's ctx transposes staged one iteration ahead so the
            # PSUM->SBUF copy and exp latency never stall the PE.
            stage_transposes(0)
            for g in range(N_G):
                # G^T for this group (ctxT[g] copied last iteration)
                st_ps = ps_big.tile([128, 512], f32, tag="big")
                nc.tensor.matmul(
                    st_ps[:],
                    qw3T[:],
                    ctxT[:, g * 512 : (g + 1) * 512],
                    start=True,
                    stop=True,
                )
                if g + 1 < N_G:
                    stage_transposes(g + 1)
                nc.scalar.activation(
                    out=e_t[:, g * 512 : (g + 1) * 512],
                    in_=st_ps[:],
                    func=AFT.Exp,
                    bias=beta_col[:],
                    scale=1.0,
                )
                # cq PSUM allocs; alpha columns ride in cq2a cols 258:262
                cq2a = ps_cq.tile([128, 262], f32, tag="cq")
                for jj in range(4):
                    i = 4 * g + jj
                    nc.tensor.matmul(
                        cq2a[:, 258 + jj : 259 + jj],
                        ctxT[:, i * 128 : (i + 1) * 128],
                        w1_col[:],
                        start=True,
                        stop=True,
                    )
                # u chain for the previous group (e_m ready by then)
                if g > 0:
                    for i in range(4 * (g - 1), 4 * g):
                        nc.tensor.matmul(
                            tail_ps[:, 0:1],
                            ctx_blk(i),
                            e_m[:, i : i + 1],
                            start=(i == 0),
                            stop=False,
                        )
                etr = ps_etr.tile([128, 512], bf16, tag="etr")
                rzs = {}
                cq_slots = {}
                cq2 = cq2a
                for j in range(4):
                    i = 4 * g + j
                    if j == 2:
                        cq2 = ps_cq.tile([128, 262], f32, tag="cq")
                    cqs = cq2[:, 129 * (j % 2) : 129 * (j % 2) + 129]
                    cq_slots[j] = cqs
                    et_sl = e_t[:, i * 128 : (i + 1) * 128]
                    nc.tensor.matmul(cqs, et_sl, rhs_aug[:], start=True, stop=True)
                    nc.tensor.transpose(
                        etr[:, j * 128 : (j + 1) * 128], et_sl, ident_bf[:]
                    )
                    if j % 2 == 1:
                        # one reciprocal per pair: Z cols sit at 128 and 257
                        rz2 = cols.tile([128, 2], f32, tag="rz")
                        zv = cq2[:, 0:258].rearrange("p (k n) -> p k n", k=2)[
                            :, :, 128
                        ]
                        nc.vector.reciprocal(rz2[:], zv)
                        rzs[j - 1] = rz2[:, 0:1]
                        rzs[j] = rz2[:, 1:2]
                for j in range(4):
                    i = 4 * g + j
                    # c2q = (E @ [qry|1]) / Z  (ACT copy with per-partition scale)
                    nc.scalar.activation(
                        out=o_blk(i, 1),
                        in_=cq_slots[j][:, 0:128],
                        func=AFT.Copy,
                        scale=rzs[j],
                    )
                # e^alpha for this group's 4 tiles (not on the store path)
                nc.scalar.activation(
                    out=e_alpha[:, 4 * g : 4 * g + 4],
                    in_=cq2a[:, 258:262],
                    func=AFT.Exp,
                )
                # row max over q of the 4 transposed tiles, then e_m
                maxE = cols.tile([128, 4], f32, tag="maxE")
                nc.vector.reduce_max(
                    out=maxE[:], in_=etr.rearrange("p (j q) -> p j q", j=4), axis=AX
                )
                nc.vector.tensor_mul(
                    e_m[:, 4 * g : 4 * g + 4], e_alpha[:, 4 * g : 4 * g + 4], maxE[:]
                )
                # out3 = ctx * c2q (SBUF-only: gpsimd can help)
                for j in range(4):
                    i = 4 * g + j
                    eng = nc.gpsimd if j >= 2 else nc.vector
                    eng.tensor_mul(o_blk(i, 2), ctx_blk(i), o_blk(i, 1))
                # this group's [c2q | ctx*c2q] is final: ship it
                # (batch 0 group 0 in two halves so DMA starts sooner)
                if b == 0 and g == 0:
                    nc.sync.dma_start(
                        out=out_v[b][:, 0:2, 128:384], in_=gv[:, 0:2, 128:384]
                    )
                    nc.sync.dma_start(
                        out=out_v[b][:, 2:4, 128:384], in_=gv[:, 2:4, 128:384]
                    )
                else:
                    nc.sync.dma_start(
                        out=out_v[b][:, 4 * g : 4 * g + 4, 128:384],
                        in_=gv[:, 4 * g : 4 * g + 4, 128:384],
                    )
            for i in range(4 * (N_G - 1), N_CT):
                nc.tensor.matmul(
                    tail_ps[:, 0:1],
                    ctx_blk(i),
                    e_m[:, i : i + 1],
                    start=False,
                    stop=(i == N_CT - 1),
                )

            # ---------- q2c epilogue ----------
            zsum = cols.tile([128, 1], f32, tag="zsum")
            nc.vector.reduce_sum(out=zsum[:], in_=e_m[:], axis=AX)
            nc.tensor.matmul(
                tail_ps[0:1, 1:2], zsum[:], ones_col[:], start=True, stop=True
            )
            u_sb = cols.tile([128, 1], f32, tag="usb")
            nc.vector.tensor_copy(u_sb[:], tail_ps[:, 0:1])
            nc.tensor.transpose(tail_ps[0:1, 2:130], u_sb[:], ident[:])
            rzb = cols.tile([1, 1], f32, tag="rzb")
            nc.vector.reciprocal(rzb[:], tail_ps[0:1, 1:2])
            q2c_row = cols.tile([1, 128], bf16, tag="q2crow")
            nc.scalar.activation(
                out=q2c_row[:], in_=tail_ps[0:1, 2:130], func=AFT.Copy, scale=rzb[:]
            )
            nc.tensor.matmul(
                tail_ps[:, 130:258], ones_row_bf[:], q2c_row[:], start=True, stop=True
            )
            q2c_sb = meds.tile([128, 128], f32, tag="q2csb")
            nc.vector.tensor_copy(q2c_sb[:], tail_ps[:, 130:258])

            # ---------- out4 + remaining stores (4 pieces) ----------
            # 3 DVE + 1 gpsimd mul per piece; each piece ships on completion
            for piece in range(4):
                for j in range(4):
                    i = 4 * piece + j
                    eng = nc.gpsimd if j == 1 else nc.vector
                    eng.tensor_mul(o_blk(i, 3), ctx_blk(i), q2c_sb[:])
                nc.sync.dma_start(
                    out=out_v[b][:, 4 * piece : 4 * piece + 4, 384:512],
                    in_=gv[:, 4 * piece : 4 * piece + 4, 384:512],
                )
            # ctx3 rides behind batch 1's stores (not needed until then)
            if b == 1:
                load_ctx(3, quarters=False)


def kernel(**inputs):
    global _compiled
    from concourse.bass_utils import run_bass_kernel_spmd

    context = np.ascontiguousarray(inputs["context"], dtype=np.float32)
    query = np.ascontiguousarray(inputs["query"], dtype=np.float32)
    w = np.ascontiguousarray(inputs["w"], dtype=np.float32)
    qmask = np.ascontiguousarray(inputs["query_mask"], dtype=np.int32)

    if _compiled is None:
        _compiled = _build()
    nc = _compiled

    core_ids = list(range(N_CORES))
    in_maps = []
    for k in core_ids:
        sl = slice(k * B_LOCAL, (k + 1) * B_LOCAL)
        in_maps.append(
            {
                "context": context[sl],
                "query": query[sl],
                "w": w,
                "query_mask": qmask[sl],
            }
        )

    res = run_bass_kernel_spmd(nc, in_maps, core_ids)
    outs = [res.results[k]["out"] for k in range(N_CORES)]
    return np.concatenate(outs, axis=0)
